# revision 10
# baseline (speedup 1.0000x reference)
"""Trainium2 Bass kernel for CRF negative log-likelihood (nn_BiLSTM_CRF).

Strategy (data-parallel over batch, 8 NeuronCores):
  - The forward-algorithm scan runs in LINEAR space:
        z_{t+1} = ef_t o (E @ z_t),   ef_t = exp(feat_t - DELTA), E = exp(trans)
    so each step is one PE matmul (block-diag E, 4 groups of 32 tags packed
    into 128 partitions) plus one VectorE elementwise multiply — no per-step
    transcendentals.
  - The scan operator is linear in the state, so the sequence is split into
    two INDEPENDENT chains that the engines pipeline against each other:
    a forward chain over t=0..511 from the START vector, and a backward
    (adjoint) chain over t=1023..512 from the STOP vector. The result is
    joined with one dot product: alpha = <r_512, z_512>.
  - Numerical range is kept by one per-sequence renormalization per chain
    (colsum via ones-matmul, reciprocal broadcast via matmul), accumulating
    log-offsets C.
  - Emissions are exponentiated/laid out on the host, streamed as bf16;
    the gold path score (pure gather over tags) is computed on the host.
"""

import os
import sys

import numpy as np

for _p in ("/opt/trn_rl_repo", "/root/.axon_site/_ro/trn_rl_repo"):
    if os.path.isdir(_p) and _p not in sys.path:
        sys.path.insert(0, _p)

import ml_dtypes

B, L, T = 4096, 1024, 32
START, STOP = T - 2, T - 1
NCORES = 8
BS = B // NCORES          # 512 sequences per core
G = 4                     # tag-groups packed into 128 partitions
F = BS // G               # 128 batch elements per group (free width)
P = G * T                 # 128 partitions
DELTA = 4.3               # per-step log-growth compensation
NSL = L // 2              # 512 slots; each slot advances both chains
RENORM = 128              # renormalize each chain every 128 slots
NREN = 2 * (NSL // RENORM - 1)  # u dumps: slots 127,255,383 x 2 chains
CH = 16                   # scan steps per DMA chunk
NCH = NSL // CH           # 32 chunks per stream
BF16 = ml_dtypes.bfloat16
ROUTE = (0, 1)            # act-route num/den of per-step multiplies (ACT copies measure ~3x formula: keep 0)
BUFS = {"ef": 3, "zf": 3, "mb": 3, "yf": 2, "yb": 2, "ps_small": 2}

_COMPILED = {}


def _build_graph(nsl=NSL, debug=False, dump_state=False):
    import concourse.mybir as mybir
    from concourse import bacc, tile

    nc = bacc.Bacc("TRN2", target_bir_lowering=False, debug=debug)
    nch = nsl // CH
    dt = mybir.dt

    eff_d = nc.dram_tensor("eff", [nch, P, CH * F], dt.bfloat16, kind="ExternalInput")
    efb_d = nc.dram_tensor("efb", [nch, P, CH * F], dt.bfloat16, kind="ExternalInput")
    z0_d = nc.dram_tensor("z0", [P, F], dt.bfloat16, kind="ExternalInput")
    r0_d = nc.dram_tensor("r0", [P, F], dt.bfloat16, kind="ExternalInput")
    ebdf_d = nc.dram_tensor("ebdf", [P, P], dt.bfloat16, kind="ExternalInput")
    ebdb_d = nc.dram_tensor("ebdb", [P, P], dt.bfloat16, kind="ExternalInput")
    ones_d = nc.dram_tensor("ones_lhsT", [P, G], dt.bfloat16, kind="ExternalInput")
    sel_d = nc.dram_tensor("sel_lhsT", [G, P], dt.bfloat16, kind="ExternalInput")
    out_d = nc.dram_tensor("out", [G, F], dt.float32, kind="ExternalOutput")
    u_out_d = nc.dram_tensor("u_out", [NREN, G, F], dt.float32, kind="ExternalOutput")
    ren_idx = [0]

    FT = mybir.ActivationFunctionType

    with tile.TileContext(nc) as tc:
        with (
            tc.tile_pool(name="const", bufs=1) as cpool,
            tc.tile_pool(name="ef", bufs=BUFS["ef"]) as efpool,
            tc.tile_pool(name="zf", bufs=BUFS["zf"]) as zfpool,
            tc.tile_pool(name="mb", bufs=BUFS["mb"]) as mbpool,
            tc.tile_pool(name="small", bufs=2) as spool,
            tc.tile_pool(name="yf", bufs=BUFS["yf"], space="PSUM") as yfpool,
            tc.tile_pool(name="yb", bufs=BUFS["yb"], space="PSUM") as ybpool,
            tc.tile_pool(name="ps_small", bufs=BUFS["ps_small"], space="PSUM") as pspool,
            tc.tile_pool(name="ps_bc", bufs=1, space="PSUM") as bcpool,
        ):
            ebdf = cpool.tile([P, P], dt.bfloat16, tag="ebdf")
            nc.sync.dma_start(ebdf[:], ebdf_d[:])
            ebdb = cpool.tile([P, P], dt.bfloat16, tag="ebdb")
            nc.sync.dma_start(ebdb[:], ebdb_d[:])
            ones_l = cpool.tile([P, G], dt.bfloat16, tag="ones")
            nc.sync.dma_start(ones_l[:], ones_d[:])
            sel_l = cpool.tile([G, P], dt.bfloat16, tag="sel")
            nc.sync.dma_start(sel_l[:], sel_d[:])

            zf = cpool.tile([P, F], dt.bfloat16, tag="zinit")
            nc.sync.dma_start(zf[:], z0_d[:])
            rb_sb = cpool.tile([P, F], dt.bfloat16, tag="rinit")
            nc.sync.dma_start(rb_sb[:], r0_d[:])
            rb_ps = None  # backward state: SBUF first slot, PSUM afterwards

            def renorm_fwd(z):
                u = pspool.tile([G, F], dt.float32, tag="u")
                nc.tensor.matmul(u[:], ones_l[:], z[:])
                r = spool.tile([G, F], dt.bfloat16, tag="r")
                with nc.allow_low_precision(reason="renorm factor"):
                    nc.vector.reciprocal(r[:], u[:])
                rbc = bcpool.tile([P, F], dt.float32, tag="rb")
                nc.tensor.matmul(rbc[:], sel_l[:], r[:])
                zn = zfpool.tile([P, F], dt.bfloat16, tag="zf")
                nc.vector.tensor_mul(zn[:], rbc[:], z[:])
                uc = spool.tile([G, F], dt.float32, tag="lnu")
                nc.scalar.copy(uc[:], u[:])
                nc.sync.dma_start(u_out_d[ren_idx[0]], uc[:])
                ren_idx[0] += 1
                return zn

            for ch in range(nch):
                eff_t = efpool.tile([P, CH * F], dt.bfloat16, tag="eff")
                efb_t = efpool.tile([P, CH * F], dt.bfloat16, tag="efb")
                w = CH * F // 4
                for q in range(4):
                    nc.sync.dma_start(
                        eff_t[:, q * w : (q + 1) * w],
                        eff_d[ch, :, q * w : (q + 1) * w],
                    )
                    nc.sync.dma_start(
                        efb_t[:, q * w : (q + 1) * w],
                        efb_d[ch, :, q * w : (q + 1) * w],
                    )
                for s in range(CH):
                    sl = ch * CH + s

                    def mul_route(dst_pool, dst_tag, src_ps, ef_ap, idx):
                        # Balance the per-step multiply across DVE and ACT:
                        # 'direct' = DVE mul straight from fp32 PSUM (1x mode);
                        # 'act'    = ScalarE copies PSUM->SBUF bf16, DVE then
                        #            muls bf16 SBUF x SBUF at 2x mode.
                        out = dst_pool.tile([P, F], dt.bfloat16, tag=dst_tag)
                        if (idx % ROUTE[1]) < ROUTE[0]:
                            yc = dst_pool.tile([P, F], dt.bfloat16, tag=dst_tag + "c")
                            nc.scalar.copy(yc[:], src_ps[:])
                            nc.vector.tensor_mul(out[:], yc[:], ef_ap)
                        else:
                            nc.vector.tensor_mul(out[:], src_ps[:], ef_ap)
                        return out

                    # ---- forward chain: y = E_f @ z ; z' = y o ef ----
                    yf = yfpool.tile([P, F], dt.float32, tag="yf")
                    nc.tensor.matmul(yf[:], ebdf[:], zf[:])
                    zf = mul_route(
                        zfpool, "zf", yf, eff_t[:, s * F : (s + 1) * F], 2 * sl
                    )
                    # ---- backward chain: m = r o ef ; r' = E_b @ m ----
                    if rb_ps is None:
                        mb = mbpool.tile([P, F], dt.bfloat16, tag="mb")
                        nc.vector.tensor_mul(
                            mb[:], rb_sb[:], efb_t[:, s * F : (s + 1) * F]
                        )
                    else:
                        mb = mul_route(
                            mbpool, "mb", rb_ps, efb_t[:, s * F : (s + 1) * F],
                            2 * sl + 1,
                        )
                    if (sl + 1) % RENORM == 0 and sl != nsl - 1:
                        zf = renorm_fwd(zf)
                        # backward renorm on m (pre-matmul; linear, so
                        # scaling here scales the whole chain)
                        u = pspool.tile([G, F], dt.float32, tag="u")
                        nc.tensor.matmul(u[:], ones_l[:], mb[:])
                        r = spool.tile([G, F], dt.bfloat16, tag="r")
                        with nc.allow_low_precision(reason="renorm factor"):
                            nc.vector.reciprocal(r[:], u[:])
                        rbc = bcpool.tile([P, F], dt.float32, tag="rb")
                        nc.tensor.matmul(rbc[:], sel_l[:], r[:])
                        mn = mbpool.tile([P, F], dt.bfloat16, tag="mb")
                        nc.vector.tensor_mul(mn[:], rbc[:], mb[:])
                        mb = mn
                        uc = spool.tile([G, F], dt.float32, tag="lnu")
                        nc.scalar.copy(uc[:], u[:])
                        nc.sync.dma_start(u_out_d[ren_idx[0]], uc[:])
                        ren_idx[0] += 1
                    rb_ps = ybpool.tile([P, F], dt.float32, tag="yb")
                    nc.tensor.matmul(rb_ps[:], ebdb[:], mb[:])

            if dump_state:
                zf_out = nc.dram_tensor("zf_out", [P, F], dt.float32, kind="ExternalOutput")
                rb_out = nc.dram_tensor("rb_out", [P, F], dt.float32, kind="ExternalOutput")
                zfc = spool.tile([P, F], dt.float32, tag="dumpz")
                nc.vector.tensor_copy(zfc[:], zf[:])
                nc.sync.dma_start(zf_out[:], zfc[:])
                rbc2 = spool.tile([P, F], dt.float32, tag="dumpr")
                nc.vector.tensor_copy(rbc2[:], rb_ps[:])
                nc.sync.dma_start(rb_out[:], rbc2[:])
            # ---- join: alpha = ln(sum_p z_512 o r_512) + C + DELTA*L ----
            q = mbpool.tile([P, F], dt.bfloat16, tag="mb")
            nc.vector.tensor_mul(q[:], rb_ps[:], zf[:])
            a = pspool.tile([G, F], dt.float32, tag="u")
            nc.tensor.matmul(a[:], ones_l[:], q[:])
            res = spool.tile([G, F], dt.float32, tag="res")
            nc.scalar.copy(res[:], a[:])
            nc.sync.dma_start(out_d[:], res[:])

    nc.compile()
    return nc


def _host_gold(feats, transitions, tags):
    tags = np.asarray(tags).astype(np.int64)
    trans = np.asarray(transitions).astype(np.float64)
    b = tags.shape[0]
    tags_ext = np.concatenate([np.full((b, 1), START, dtype=np.int64), tags], axis=1)
    trans_score = trans[tags_ext[:, 1:], tags_ext[:, :-1]].sum(axis=1)
    emit = np.take_along_axis(
        np.asarray(feats).astype(np.float64), tags[:, :, None], axis=2
    )[:, :, 0].sum(axis=1)
    return trans_score + emit + trans[STOP, tags[:, -1]]


def _chunk(x):
    # [NCORES, NSL, P, F] -> [NCORES, NCH, P, CH*F]
    x = x.reshape(NCORES, NCH, CH, P, F).transpose(0, 1, 3, 2, 4)
    return np.ascontiguousarray(x).reshape(NCORES, NCH, P, CH * F)


def prepare_inputs(feats, transitions, tags):
    feats = np.asarray(feats, dtype=np.float32)
    trans = np.asarray(transitions, dtype=np.float32)
    gold = _host_gold(feats, transitions, tags)

    # arr[c, t, g*32+p, j] = exp(feats[c*512 + g*128 + j, t, p] - DELTA), bf16
    arr = np.exp(feats - DELTA).astype(BF16)
    arr = arr.reshape(NCORES, G, F, L, T).transpose(0, 3, 1, 4, 2)
    arr = np.ascontiguousarray(arr).reshape(NCORES, L, P, F)
    eff = _chunk(arr[:, :NSL])
    efb = _chunk(arr[:, : NSL - 1 : -1])  # t = 1023 down to 512

    z0 = np.zeros((P, F), dtype=BF16)
    z0[START::T, :] = 1.0
    estop_col = np.exp(trans[STOP].astype(np.float64)).astype(np.float32)
    r0 = np.zeros((P, F), dtype=np.float32)
    for g in range(G):
        r0[g * T : (g + 1) * T, :] = estop_col[:, None]
    r0 = r0.astype(BF16)

    E = np.exp(trans.astype(np.float64)).astype(np.float32)
    ebdf = np.zeros((P, P), dtype=np.float32)
    ebdb = np.zeros((P, P), dtype=np.float32)
    for g in range(G):
        ebdf[g * T : (g + 1) * T, g * T : (g + 1) * T] = E.T
        ebdb[g * T : (g + 1) * T, g * T : (g + 1) * T] = E
    ebdf = ebdf.astype(BF16)
    ebdb = ebdb.astype(BF16)

    ones_l = np.zeros((P, G), dtype=BF16)
    sel_l = np.zeros((G, P), dtype=BF16)
    for g in range(G):
        ones_l[g * T : (g + 1) * T, g] = 1.0
        sel_l[g, g * T : (g + 1) * T] = 1.0

    in_maps = [
        {
            "eff": eff[c],
            "efb": efb[c],
            "z0": z0,
            "r0": r0,
            "ebdf": ebdf,
            "ebdb": ebdb,
            "ones_lhsT": ones_l,
            "sel_lhsT": sel_l,
        }
        for c in range(NCORES)
    ]
    return {"in_maps": in_maps, "gold": gold}


def finalize(results, prep):
    alpha_parts = []
    for c in range(NCORES):
        a = np.log(results[c]["out"].astype(np.float64))
        a += np.log(results[c]["u_out"].astype(np.float64)).sum(axis=0)
        alpha_parts.append(a.reshape(BS))
    alpha = np.concatenate(alpha_parts) + DELTA * L
    return (alpha - prep["gold"]).astype(np.float32)


def kernel(feats, transitions, tags):
    from concourse.bass_utils import run_bass_kernel_spmd

    prep = prepare_inputs(feats, transitions, tags)
    if "graph" not in _COMPILED:
        _COMPILED["graph"] = _build_graph()
    nc = _COMPILED["graph"]
    res = run_bass_kernel_spmd(nc, prep["in_maps"], core_ids=list(range(NCORES)))
    global _LAST_RESULTS
    _LAST_RESULTS = res
    return finalize(res.results, prep)


# revision 11
# speedup vs baseline: 2.3063x; 2.3063x over previous
"""Trainium2 Bass kernel for CRF negative log-likelihood (nn_BiLSTM_CRF).

Strategy (data-parallel over batch, 8 NeuronCores):
  - The forward-algorithm scan runs in LINEAR space:
        z_{t+1} = ef_t o (E @ z_t),   ef_t = exp(feat_t - DELTA), E = exp(trans)
    so each step is one PE matmul (block-diag E, 4 groups of 32 tags packed
    into 128 partitions) plus one VectorE elementwise multiply — no per-step
    transcendentals.
  - The scan operator is linear in the state, so the sequence is split into
    two INDEPENDENT chains that the engines pipeline against each other:
    a forward chain over t=0..511 from the START vector, and a backward
    (adjoint) chain over t=1023..512 from the STOP vector. The result is
    joined with one dot product: alpha = <r_512, z_512>.
  - Numerical range is kept by one per-sequence renormalization per chain
    (colsum via ones-matmul, reciprocal broadcast via matmul), accumulating
    log-offsets C.
  - Emissions are exponentiated/laid out on the host, streamed as bf16;
    the gold path score (pure gather over tags) is computed on the host.
"""

import os
import sys

import numpy as np

for _p in ("/opt/trn_rl_repo", "/root/.axon_site/_ro/trn_rl_repo"):
    if os.path.isdir(_p) and _p not in sys.path:
        sys.path.insert(0, _p)

import ml_dtypes

B, L, T = 4096, 1024, 32
START, STOP = T - 2, T - 1
NCORES = 8
BS = B // NCORES          # 512 sequences per core
G = 4                     # tag-groups packed into 128 partitions
F = BS // G               # 128 batch elements per group (free width)
P = G * T                 # 128 partitions
DELTA = 4.3               # per-step log-growth compensation
NSL = L // 2              # 512 slots; each slot advances both chains
RENORM = 128              # renormalize each chain every 128 slots
NREN = 2 * (NSL // RENORM - 1)  # u dumps: slots 127,255,383 x 2 chains
CH = 16                   # scan steps per DMA chunk
NCH = NSL // CH           # 32 chunks per stream
BF16 = ml_dtypes.bfloat16
ROUTE = (0, 1)            # act-route num/den of per-step multiplies (ACT copies measure ~3x formula: keep 0)
BUFS = {"ef": 3, "zf": 3, "mb": 3, "yf": 2, "yb": 2, "ps_small": 2}

_COMPILED = {}


def _build_graph(nsl=NSL, debug=False, dump_state=False, repeat=0):
    import concourse.mybir as mybir
    from concourse import bacc, tile

    nc = bacc.Bacc("TRN2", target_bir_lowering=False, debug=debug)
    nch = nsl // CH
    dt = mybir.dt

    eff_d = nc.dram_tensor("eff", [nch, P, CH * F], dt.bfloat16, kind="ExternalInput")
    efb_d = nc.dram_tensor("efb", [nch, P, CH * F], dt.bfloat16, kind="ExternalInput")
    z0_d = nc.dram_tensor("z0", [P, F], dt.bfloat16, kind="ExternalInput")
    r0_d = nc.dram_tensor("r0", [P, F], dt.bfloat16, kind="ExternalInput")
    ebdf_d = nc.dram_tensor("ebdf", [P, P], dt.bfloat16, kind="ExternalInput")
    ebdb_d = nc.dram_tensor("ebdb", [P, P], dt.bfloat16, kind="ExternalInput")
    ones_d = nc.dram_tensor("ones_lhsT", [P, G], dt.bfloat16, kind="ExternalInput")
    sel_d = nc.dram_tensor("sel_lhsT", [G, P], dt.bfloat16, kind="ExternalInput")
    out_d = nc.dram_tensor("out", [G, F], dt.float32, kind="ExternalOutput")
    u_out_d = nc.dram_tensor("u_out", [NREN, G, F], dt.float32, kind="ExternalOutput")
    ren_idx = [0]

    FT = mybir.ActivationFunctionType

    with tile.TileContext(nc) as tc:
        with (
            tc.tile_pool(name="const", bufs=1) as cpool,
            tc.tile_pool(name="ef", bufs=BUFS["ef"]) as efpool,
            tc.tile_pool(name="zf", bufs=BUFS["zf"]) as zfpool,
            tc.tile_pool(name="mb", bufs=BUFS["mb"]) as mbpool,
            tc.tile_pool(name="small", bufs=2) as spool,
            tc.tile_pool(name="yf", bufs=BUFS["yf"], space="PSUM") as yfpool,
            tc.tile_pool(name="yb", bufs=BUFS["yb"], space="PSUM") as ybpool,
            tc.tile_pool(name="ps_small", bufs=BUFS["ps_small"], space="PSUM") as pspool,
            tc.tile_pool(name="ps_bc", bufs=1, space="PSUM") as bcpool,
        ):
            ebdf = cpool.tile([P, P], dt.bfloat16, tag="ebdf")
            nc.sync.dma_start(ebdf[:], ebdf_d[:])
            ebdb = cpool.tile([P, P], dt.bfloat16, tag="ebdb")
            nc.sync.dma_start(ebdb[:], ebdb_d[:])
            ones_l = cpool.tile([P, G], dt.bfloat16, tag="ones")
            nc.sync.dma_start(ones_l[:], ones_d[:])
            sel_l = cpool.tile([G, P], dt.bfloat16, tag="sel")
            nc.sync.dma_start(sel_l[:], sel_d[:])

            import contextlib
            rep_cm = tc.For_i(0, repeat, 1) if repeat else contextlib.nullcontext()
            rep_cm.__enter__()
            zf = cpool.tile([P, F], dt.bfloat16, tag="zinit")
            nc.sync.dma_start(zf[:], z0_d[:])
            rb_sb = cpool.tile([P, F], dt.bfloat16, tag="rinit")
            nc.sync.dma_start(rb_sb[:], r0_d[:])
            rb_ps = None  # backward state: SBUF first slot, PSUM afterwards

            def renorm_fwd(z):
                u = pspool.tile([G, F], dt.float32, tag="u")
                nc.tensor.matmul(u[:], ones_l[:], z[:])
                r = spool.tile([G, F], dt.bfloat16, tag="r")
                with nc.allow_low_precision(reason="renorm factor"):
                    nc.vector.reciprocal(r[:], u[:])
                rbc = bcpool.tile([P, F], dt.float32, tag="rb")
                nc.tensor.matmul(rbc[:], sel_l[:], r[:])
                zn = zfpool.tile([P, F], dt.bfloat16, tag="zf")
                nc.vector.tensor_mul(zn[:], rbc[:], z[:])
                uc = spool.tile([G, F], dt.float32, tag="lnu")
                nc.scalar.copy(uc[:], u[:])
                nc.sync.dma_start(u_out_d[ren_idx[0]], uc[:])
                ren_idx[0] += 1
                return zn

            for ch in range(nch):
                eff_t = efpool.tile([P, CH * F], dt.bfloat16, tag="eff")
                efb_t = efpool.tile([P, CH * F], dt.bfloat16, tag="efb")
                w = CH * F // 4
                for q in range(4):
                    nc.sync.dma_start(
                        eff_t[:, q * w : (q + 1) * w],
                        eff_d[ch, :, q * w : (q + 1) * w],
                    )
                    nc.sync.dma_start(
                        efb_t[:, q * w : (q + 1) * w],
                        efb_d[ch, :, q * w : (q + 1) * w],
                    )
                for s in range(CH):
                    sl = ch * CH + s

                    def mul_route(dst_pool, dst_tag, src_ps, ef_ap, idx):
                        # Balance the per-step multiply across DVE and ACT:
                        # 'direct' = DVE mul straight from fp32 PSUM (1x mode);
                        # 'act'    = ScalarE copies PSUM->SBUF bf16, DVE then
                        #            muls bf16 SBUF x SBUF at 2x mode.
                        out = dst_pool.tile([P, F], dt.bfloat16, tag=dst_tag)
                        if (idx % ROUTE[1]) < ROUTE[0]:
                            yc = dst_pool.tile([P, F], dt.bfloat16, tag=dst_tag + "c")
                            nc.scalar.copy(yc[:], src_ps[:])
                            nc.vector.tensor_mul(out[:], yc[:], ef_ap)
                        else:
                            nc.vector.tensor_mul(out[:], src_ps[:], ef_ap)
                        return out

                    # ---- forward chain: y = E_f @ z ; z' = y o ef ----
                    yf = yfpool.tile([P, F], dt.float32, tag="yf")
                    nc.tensor.matmul(yf[:], ebdf[:], zf[:])
                    zf = mul_route(
                        zfpool, "zf", yf, eff_t[:, s * F : (s + 1) * F], 2 * sl
                    )
                    # ---- backward chain: m = r o ef ; r' = E_b @ m ----
                    if rb_ps is None:
                        mb = mbpool.tile([P, F], dt.bfloat16, tag="mb")
                        nc.vector.tensor_mul(
                            mb[:], rb_sb[:], efb_t[:, s * F : (s + 1) * F]
                        )
                    else:
                        mb = mul_route(
                            mbpool, "mb", rb_ps, efb_t[:, s * F : (s + 1) * F],
                            2 * sl + 1,
                        )
                    if (sl + 1) % RENORM == 0 and sl != nsl - 1:
                        zf = renorm_fwd(zf)
                        # backward renorm on m (pre-matmul; linear, so
                        # scaling here scales the whole chain)
                        u = pspool.tile([G, F], dt.float32, tag="u")
                        nc.tensor.matmul(u[:], ones_l[:], mb[:])
                        r = spool.tile([G, F], dt.bfloat16, tag="r")
                        with nc.allow_low_precision(reason="renorm factor"):
                            nc.vector.reciprocal(r[:], u[:])
                        rbc = bcpool.tile([P, F], dt.float32, tag="rb")
                        nc.tensor.matmul(rbc[:], sel_l[:], r[:])
                        mn = mbpool.tile([P, F], dt.bfloat16, tag="mb")
                        nc.vector.tensor_mul(mn[:], rbc[:], mb[:])
                        mb = mn
                        uc = spool.tile([G, F], dt.float32, tag="lnu")
                        nc.scalar.copy(uc[:], u[:])
                        nc.sync.dma_start(u_out_d[ren_idx[0]], uc[:])
                        ren_idx[0] += 1
                    rb_ps = ybpool.tile([P, F], dt.float32, tag="yb")
                    nc.tensor.matmul(rb_ps[:], ebdb[:], mb[:])

            if dump_state:
                zf_out = nc.dram_tensor("zf_out", [P, F], dt.float32, kind="ExternalOutput")
                rb_out = nc.dram_tensor("rb_out", [P, F], dt.float32, kind="ExternalOutput")
                zfc = spool.tile([P, F], dt.float32, tag="dumpz")
                nc.vector.tensor_copy(zfc[:], zf[:])
                nc.sync.dma_start(zf_out[:], zfc[:])
                rbc2 = spool.tile([P, F], dt.float32, tag="dumpr")
                nc.vector.tensor_copy(rbc2[:], rb_ps[:])
                nc.sync.dma_start(rb_out[:], rbc2[:])
            # ---- join: alpha = ln(sum_p z_512 o r_512) + C + DELTA*L ----
            q = mbpool.tile([P, F], dt.bfloat16, tag="mb")
            nc.vector.tensor_mul(q[:], rb_ps[:], zf[:])
            a = pspool.tile([G, F], dt.float32, tag="u")
            nc.tensor.matmul(a[:], ones_l[:], q[:])
            res = spool.tile([G, F], dt.float32, tag="res")
            nc.scalar.copy(res[:], a[:])
            nc.sync.dma_start(out_d[:], res[:])
            rep_cm.__exit__(None, None, None)

    nc.compile()
    return nc


def _host_gold(feats, transitions, tags):
    tags = np.asarray(tags).astype(np.int64)
    trans = np.asarray(transitions).astype(np.float64)
    b = tags.shape[0]
    tags_ext = np.concatenate([np.full((b, 1), START, dtype=np.int64), tags], axis=1)
    trans_score = trans[tags_ext[:, 1:], tags_ext[:, :-1]].sum(axis=1)
    emit = np.take_along_axis(
        np.asarray(feats).astype(np.float64), tags[:, :, None], axis=2
    )[:, :, 0].sum(axis=1)
    return trans_score + emit + trans[STOP, tags[:, -1]]


def _chunk(x):
    # [NCORES, NSL, P, F] -> [NCORES, NCH, P, CH*F]
    x = x.reshape(NCORES, NCH, CH, P, F).transpose(0, 1, 3, 2, 4)
    return np.ascontiguousarray(x).reshape(NCORES, NCH, P, CH * F)


def prepare_inputs(feats, transitions, tags):
    feats = np.asarray(feats, dtype=np.float32)
    trans = np.asarray(transitions, dtype=np.float32)
    gold = _host_gold(feats, transitions, tags)

    # arr[c, t, g*32+p, j] = exp(feats[c*512 + g*128 + j, t, p] - DELTA), bf16
    arr = np.exp(feats - DELTA).astype(BF16)
    arr = arr.reshape(NCORES, G, F, L, T).transpose(0, 3, 1, 4, 2)
    arr = np.ascontiguousarray(arr).reshape(NCORES, L, P, F)
    eff = _chunk(arr[:, :NSL])
    efb = _chunk(arr[:, : NSL - 1 : -1])  # t = 1023 down to 512

    z0 = np.zeros((P, F), dtype=BF16)
    z0[START::T, :] = 1.0
    estop_col = np.exp(trans[STOP].astype(np.float64)).astype(np.float32)
    r0 = np.zeros((P, F), dtype=np.float32)
    for g in range(G):
        r0[g * T : (g + 1) * T, :] = estop_col[:, None]
    r0 = r0.astype(BF16)

    E = np.exp(trans.astype(np.float64)).astype(np.float32)
    ebdf = np.zeros((P, P), dtype=np.float32)
    ebdb = np.zeros((P, P), dtype=np.float32)
    for g in range(G):
        ebdf[g * T : (g + 1) * T, g * T : (g + 1) * T] = E.T
        ebdb[g * T : (g + 1) * T, g * T : (g + 1) * T] = E
    ebdf = ebdf.astype(BF16)
    ebdb = ebdb.astype(BF16)

    ones_l = np.zeros((P, G), dtype=BF16)
    sel_l = np.zeros((G, P), dtype=BF16)
    for g in range(G):
        ones_l[g * T : (g + 1) * T, g] = 1.0
        sel_l[g, g * T : (g + 1) * T] = 1.0

    in_maps = [
        {
            "eff": eff[c],
            "efb": efb[c],
            "z0": z0,
            "r0": r0,
            "ebdf": ebdf,
            "ebdb": ebdb,
            "ones_lhsT": ones_l,
            "sel_lhsT": sel_l,
        }
        for c in range(NCORES)
    ]
    return {"in_maps": in_maps, "gold": gold}


def finalize(results, prep):
    alpha_parts = []
    for c in range(NCORES):
        a = np.log(results[c]["out"].astype(np.float64))
        a += np.log(results[c]["u_out"].astype(np.float64)).sum(axis=0)
        alpha_parts.append(a.reshape(BS))
    alpha = np.concatenate(alpha_parts) + DELTA * L
    return (alpha - prep["gold"]).astype(np.float32)


def kernel(feats, transitions, tags):
    from concourse.bass_utils import run_bass_kernel_spmd

    prep = prepare_inputs(feats, transitions, tags)
    if "graph" not in _COMPILED:
        _COMPILED["graph"] = _build_graph()
    nc = _COMPILED["graph"]
    res = run_bass_kernel_spmd(nc, prep["in_maps"], core_ids=list(range(NCORES)))
    global _LAST_RESULTS
    _LAST_RESULTS = res
    return finalize(res.results, prep)
